# revision 1
# baseline (speedup 1.0000x reference)
"""Contextual LSTM cell on 8 Trainium2 NeuronCores.

Strategy:
  - Shard the batch dim (B=65536) across 8 cores (8192 each), replicate weights.
  - All 15 gate matmuls fused into ONE (1024 x 832) @ (832 x B) matmul:
        rows:  [gate_i | gate_f | gate_c | gate_o]      (4 x 256)
        cols:  [x (256) | h (256) | c (256) | topic (64)]
    with -w_ho folded in and the (gate_c, c) block identically zero (skipped).
  - Matmul in fp16 (1 cycle/row on PE vs 4 for fp32), accumulated fp32 in PSUM.
    x/h/topic + weights are cast to fp16 on the host; c is shipped fp32 (needed
    exactly for cc = cf*c + ...) and cast to fp16 on-device for the matmul.
  - Per-gate bias is fused into the ScalarE activation (sigmoid/tanh) that
    reads the PSUM bank directly; cc/ch elementwise runs fp32 on VectorE.
"""

import os
import numpy as np

import concourse.bass as bass
import concourse.bacc as bacc
import concourse.mybir as mybir
from concourse.tile import TileContext
from concourse.bass_utils import run_bass_kernel_spmd

I, H, T, B = 256, 256, 64, 65536
NCORES = 8
BS = B // NCORES          # 8192 batch columns per core
NT = 512                  # columns per chunk (one PSUM bank of fp32)
NCHUNK = BS // NT         # 16
KB = 7                    # k-blocks: x:2, h:2, c:2, topic:1(64 rows)
MB = 8                    # m-blocks: 4 gates x 2

FP16 = mybir.dt.float16
FP32 = mybir.dt.float32
SIG = mybir.ActivationFunctionType.Sigmoid
TANH = mybir.ActivationFunctionType.Tanh

_PROGRAM = None
_LAST_RESULTS = None  # for test harness introspection


def _build_program(repeat=1):
    # Bacc (not Bass): its compile() pass splits multi-semaphore waits into
    # InstEventSemaphore preludes — walrus rejects >1 sync wait per
    # instruction otherwise.
    nc = bacc.Bacc()

    xh = nc.declare_dram_parameter("xh", [2 * H, BS], FP16, isOutput=False)
    c_in = nc.declare_dram_parameter("c", [H, BS], FP32, isOutput=False)
    topic = nc.declare_dram_parameter("topic", [T, BS], FP16, isOutput=False)
    wt = nc.declare_dram_parameter("wt", [128, KB * 1024], FP16, isOutput=False)
    biases = nc.declare_dram_parameter("biases", [128, MB], FP32, isOutput=False)
    ch_out = nc.declare_dram_parameter("ch", [H, BS], FP32, isOutput=True)
    cc_out = nc.declare_dram_parameter("cc", [H, BS], FP32, isOutput=True)

    chunks = [(i * NT, NT) for i in range(NCHUNK)]

    with TileContext(nc) as tc:
        with (
            tc.tile_pool(name="const", bufs=1) as constp,
            tc.tile_pool(name="zin", bufs=4) as zp,
            tc.tile_pool(name="gates", bufs=2) as gp,
            tc.tile_pool(name="psum", bufs=1, space="PSUM") as pp,
        ):
            wsb = constp.tile([128, KB * 1024], FP16, tag="w", name="wsb")
            bsb = constp.tile([128, MB], FP32, tag="b", name="bsb")
            # k0 weights first: the very first matmul needs only this block.
            # Remaining weight blocks + biases are interleaved between the
            # first chunk's input DMAs below (8 HWDGE queues run them in
            # parallel) so the PE can start ~2us after kernel entry.
            nc.sync.dma_start(out=wsb[:, 0:1024], in_=wt[:, 0:1024])
            pending_w = list(range(1, KB)) + [None]  # None -> bias DMA

            def weight_dma():
                if not pending_w:
                    return
                k = pending_w.pop(0)
                if k is None:
                    nc.sync.dma_start(out=bsb[:], in_=biases[:])
                else:
                    nc.sync.dma_start(out=wsb[:, k * 1024:(k + 1) * 1024],
                                      in_=wt[:, k * 1024:(k + 1) * 1024])

            # PE warm-up: ~2.5us of tiny matmuls hidden under the initial DMA
            # fill releases the HAM clock gate (cold PE runs at 1.2 GHz for
            # its first ~3.4us of activity) before the real stream begins.
            wz = constp.tile([128, 64], FP16, tag="wz", name="wz")
            nc.vector.memset(wz[:], 0.0)
            pdum = pp.tile([128, NT], FP32, tag="ps0", name="pdum")
            for _ in range(28):
                nc.tensor.matmul(pdum[0:64, 0:64], wz[:, 0:64], wz[:, 0:64],
                                 start=True, stop=True)

            for rn in range(repeat * len(chunks)):
                r, n = divmod(rn, len(chunks))
                c0, nt = chunks[n]
                c1 = c0 + nt

                # ---- load inputs for this column chunk ----
                # batched 3D-AP DMAs: one transfer for all four 128-row
                # blocks of [x; h], one for both blocks of c
                z = []
                for j in range(4):
                    zj = zp.tile([128, nt], FP16, tag=f"z{j}", name=f"z{j}_{n}")
                    nc.sync.dma_start(out=zj[:], in_=xh[j * 128:(j + 1) * 128, c0:c1])
                    if rn == 0:
                        weight_dma()
                    z.append(zj)
                cf32 = []
                for j in range(2):
                    cj = zp.tile([128, nt], FP32, tag=f"c{j}", name=f"c{j}_{n}")
                    nc.sync.dma_start(out=cj[:], in_=c_in[j * 128:(j + 1) * 128, c0:c1])
                    if rn == 0:
                        weight_dma()
                    c16 = zp.tile([128, nt], FP16, tag=f"c16_{j}", name=f"c16_{j}_{n}")
                    nc.vector.tensor_copy(out=c16[:], in_=cj[:])
                    cf32.append(cj[:])
                    z.append(c16)
                # topic duplicated into partitions 64-127 so the two topic
                # matmuls of an m-pair can run concurrently via row packing
                tp = zp.tile([128, nt], FP16, tag="tp", name=f"tp_{n}")
                nc.sync.dma_start(out=tp[0:T, :], in_=topic[:, c0:c1])
                if rn == 0:
                    weight_dma()
                nc.sync.dma_start(out=tp[T:128, :], in_=topic[:, c0:c1])
                while rn == 0 and pending_w:
                    weight_dma()

                # ---- the fused gate matmul ----
                # Per m-pair: all K=128 blocks of both m's first, then the two
                # K=64 topic matmuls back-to-back on distinct row-groups
                # ((0,0) and (64,0)) — they execute concurrently in the PE
                # array, halving the topic block's cost.
                ps = [None] * MB
                for m0 in range(0, MB, 2):
                    for m in (m0, m0 + 1):
                        pst = pp.tile([128, nt], FP32, tag=f"ps{m}", name=f"ps{m}_{n}")
                        # gate_c (m 4,5) has no c-term: skip the zero blocks
                        ks = [k for k in range(6) if not (m in (4, 5) and k in (4, 5))]
                        for i, k in enumerate(ks):
                            lhsT = wsb[:, k * 1024 + m * 128: k * 1024 + (m + 1) * 128]
                            nc.tensor.matmul(
                                pst[:], lhsT, z[k][:],
                                start=(i == 0), stop=False,
                            )
                        ps[m] = pst
                    for m in (m0, m0 + 1):
                        p0 = T * (m & 1)
                        lhsT = wsb[p0:p0 + T, 6 * 1024 + m * 128: 6 * 1024 + (m + 1) * 128]
                        nc.tensor.matmul(
                            ps[m][:], lhsT, tp[p0:p0 + T, :],
                            start=False, stop=True,
                            tile_position=(p0, 0),
                        )

                # ---- gate activations (bias fused, reads PSUM) ----
                def act(m, fn, nm):
                    t = gp.tile([128, nt], FP32, tag=nm, name=f"{nm}_{n}")
                    nc.scalar.activation(out=t[:], in_=ps[m][:], func=fn,
                                         bias=bsb[:, m:m + 1])
                    return t

                for half in range(2):
                    ci = act(0 + half, SIG, f"ci{half}")
                    cf = act(2 + half, SIG, f"cf{half}")
                    tg = act(4 + half, TANH, f"tg{half}")
                    co = act(6 + half, SIG, f"co{half}")

                    t1 = gp.tile([128, nt], FP32, tag=f"t1{half}", name=f"t1{half}_{n}")
                    nc.vector.tensor_mul(t1[:], ci[:], tg[:])
                    t2 = gp.tile([128, nt], FP32, tag=f"t2{half}", name=f"t2{half}_{n}")
                    nc.vector.tensor_mul(t2[:], cf[:], cf32[half])
                    cct = gp.tile([128, nt], FP32, tag=f"cc{half}", name=f"cc{half}_{n}")
                    nc.vector.tensor_add(cct[:], t1[:], t2[:])
                    tcc = gp.tile([128, nt], FP32, tag=f"tcc{half}", name=f"tcc{half}_{n}")
                    nc.scalar.activation(out=tcc[:], in_=cct[:], func=TANH)
                    cht = gp.tile([128, nt], FP32, tag=f"chh{half}", name=f"chh{half}_{n}")
                    nc.vector.tensor_mul(cht[:], co[:], tcc[:])
                    r0, r1 = half * 128, (half + 1) * 128
                    nc.sync.dma_start(out=cc_out[r0:r1, c0:c1], in_=cct[:])
                    nc.sync.dma_start(out=ch_out[r0:r1, c0:c1], in_=cht[:])

    nc.finalize()
    return nc


def _prep_weights(inp):
    """Assemble the fused (1024, 832) weight and return lhsT blocks + biases."""
    Wf = np.zeros((1024, 832), np.float32)

    def put(g, blocks):
        r = g * 256
        for j, wb in enumerate(blocks):
            if wb is None:
                continue
            col = j * 256
            Wf[r:r + 256, col:col + wb.shape[1]] = wb

    put(0, [inp["w_ii"], inp["w_hi"], inp["w_ci"], inp["w_bi"]])
    put(1, [inp["w_if"], inp["w_hf"], inp["w_cf"], inp["w_bf"]])
    put(2, [inp["w_ic"], inp["w_hc"], None, inp["w_bc"]])
    put(3, [inp["w_io"], -inp["w_ho"], inp["w_co"], inp["w_bo"]])

    wT = np.zeros((KB * 128, 1024), np.float32)
    wT[:832] = Wf.T
    # duplicate topic weight rows into partitions 64-127 of the k=6 block
    # (row-packed topic matmuls read them at base_partition 64)
    wT[832:896] = wT[768:832]
    # (7,128,1024) -> (128, 7*1024) with block-k contiguous in the free dim
    wt_host = np.ascontiguousarray(
        wT.reshape(KB, 128, 1024).transpose(1, 0, 2).reshape(128, KB * 1024)
    ).astype(np.float16)

    bias_all = np.concatenate(
        [inp["bias_i"], inp["bias_f"], inp["bias_c"], inp["bias_o"]], axis=0
    ).reshape(MB, 128)
    bias_host = np.ascontiguousarray(bias_all.T).astype(np.float32)
    return wt_host, bias_host


def kernel(**inputs):
    global _PROGRAM, _LAST_RESULTS
    if _PROGRAM is None:
        _PROGRAM = _build_program()
    nc = _PROGRAM

    inp = {k: np.asarray(v, dtype=np.float32) for k, v in inputs.items()}
    wt_host, bias_host = _prep_weights(inp)

    xh16 = np.concatenate([inp["x"], inp["h"]], axis=0).astype(np.float16)
    t16 = inp["topic"].astype(np.float16)
    c32 = inp["c"]

    in_maps = []
    for i in range(NCORES):
        sl = slice(i * BS, (i + 1) * BS)
        in_maps.append({
            "xh": np.ascontiguousarray(xh16[:, sl]),
            "c": np.ascontiguousarray(c32[:, sl]),
            "topic": np.ascontiguousarray(t16[:, sl]),
            "wt": wt_host,
            "biases": bias_host,
        })

    res = run_bass_kernel_spmd(
        nc, in_maps, list(range(NCORES)),
        trace=bool(os.environ.get("KERNEL_TRACE")),
    )
    _LAST_RESULTS = res

    ch = np.concatenate([res.results[i]["ch"] for i in range(NCORES)], axis=1)
    cc = np.concatenate([res.results[i]["cc"] for i in range(NCORES)], axis=1)
    return np.stack([ch, cc], axis=0)



# revision 2
# speedup vs baseline: 1.7508x; 1.7508x over previous
"""Contextual LSTM cell on 8 Trainium2 NeuronCores.

Strategy (v2, fp8 DoubleRow):
  - Shard the batch dim (B=65536) across 8 cores (8192 each), replicate weights.
  - All gate matmuls fused into one (1024 x 832) @ (832 x B) matmul:
        rows:  [gate_i | gate_f | gate_c | gate_o]      (4 x 256)
        cols:  [x (256) | h (256) | c (256) | topic (64)]
    with -w_ho folded in and the (gate_c, c) block identically zero (skipped).
  - Precision split by gate (error budget measured vs fp32 reference):
      * sigmoid gates i/f/o: fp8 e4m3 DoubleRow matmuls. Each 128-row m-block
        needs only 4 matmul instructions (x-pair, h-pair, c-pair, topic-pair
        with Ki=32), each contracting 2x128 K rows per pass.
        Weights are scaled by 64 into e4m3's normal range; the 1/64 descale is
        folded into the ScalarE activation's `scale` operand.
      * tanh candidate gate: fp16 matmuls (the tanh path has unit slope and
        dominates the quantization error budget; fp8 there breaks 2e-2).
  - Elementwise cc/ch runs fp16 on VectorE; outputs ship fp16 and are upcast
    on the host. c is shipped fp16 for the elementwise path and e4m3 for the
    matmul path.
"""

import os
import numpy as np
import ml_dtypes

import concourse.bass as bass
import concourse.bacc as bacc
import concourse.mybir as mybir
from concourse.tile import TileContext
from concourse.bass_utils import run_bass_kernel_spmd

I, H, T, B = 256, 256, 64, 65536
NCORES = 8
BS = B // NCORES          # 8192 batch columns per core
NT = 512                  # columns per chunk (one PSUM bank of fp32)
NCHUNK = BS // NT         # 16

WSCALE = 64.0             # fp8 weight pre-scale (power of 2), descaled in ACT

FP8 = mybir.dt.float8e4
FP16 = mybir.dt.float16
FP32 = mybir.dt.float32
SIG = mybir.ActivationFunctionType.Sigmoid
TANH = mybir.ActivationFunctionType.Tanh
DR = mybir.MatmulPerfMode.DoubleRow

IFO_M = [0, 1, 2, 3, 6, 7]   # m-blocks of gates i, f, o (fp8 path)
G_M = [4, 5]                 # m-blocks of the tanh candidate gate (fp16 path)

_PROGRAM = None
_LAST_RESULTS = None  # for test harness introspection


def _build_program(nchunk=NCHUNK):
    nc = bacc.Bacc()

    # --- inputs (per-core shard), laid out host-side for direct 3D-AP DMA ---
    x8d = nc.declare_dram_parameter("x8", [128, 2, nchunk * NT], FP8, isOutput=False)
    h8d = nc.declare_dram_parameter("h8", [128, 2, nchunk * NT], FP8, isOutput=False)
    c8d = nc.declare_dram_parameter("c8", [128, 2, nchunk * NT], FP8, isOutput=False)
    t8d = nc.declare_dram_parameter("t8", [32, 2, nchunk * NT], FP8, isOutput=False)
    z16d = nc.declare_dram_parameter("z16", [128, 4, nchunk * NT], FP16, isOutput=False)
    t16d = nc.declare_dram_parameter("t16", [64, nchunk * NT], FP16, isOutput=False)
    c16d = nc.declare_dram_parameter("c16", [128, 2, nchunk * NT], FP16, isOutput=False)
    # weights (DoubleRow lhsT layout [Ki, 2, 6*128] for the six i/f/o m-blocks)
    w8xd = nc.declare_dram_parameter("w8x", [128, 2, 768], FP8, isOutput=False)
    w8hd = nc.declare_dram_parameter("w8h", [128, 2, 768], FP8, isOutput=False)
    w8cd = nc.declare_dram_parameter("w8c", [128, 2, 768], FP8, isOutput=False)
    w8td = nc.declare_dram_parameter("w8t", [32, 2, 768], FP8, isOutput=False)
    w16d = nc.declare_dram_parameter("w16", [128, 4, 256], FP16, isOutput=False)
    w16td = nc.declare_dram_parameter("w16t", [64, 256], FP16, isOutput=False)
    biasd = nc.declare_dram_parameter("biases", [128, 8], FP32, isOutput=False)
    # outputs, fp16, [partition, half, col]
    ccd = nc.declare_dram_parameter("cc", [128, 2, nchunk * NT], FP16, isOutput=True)
    chd = nc.declare_dram_parameter("ch", [128, 2, nchunk * NT], FP16, isOutput=True)

    with TileContext(nc) as tc:
        with (
            tc.tile_pool(name="const", bufs=1) as constp,
            tc.tile_pool(name="zin", bufs=3) as zp,
            tc.tile_pool(name="gates", bufs=2) as gp,
            tc.tile_pool(name="psum", bufs=1, space="PSUM") as pp,
        ):
            w8x = constp.tile([128, 2, 768], FP8, tag="w8x", name="w8x")
            w8h = constp.tile([128, 2, 768], FP8, tag="w8h", name="w8h")
            w8c = constp.tile([128, 2, 768], FP8, tag="w8c", name="w8c")
            w8t = constp.tile([32, 2, 768], FP8, tag="w8t", name="w8t")
            w16 = constp.tile([128, 4, 256], FP16, tag="w16", name="w16")
            w16t = constp.tile([64, 256], FP16, tag="w16t", name="w16t")
            bsb = constp.tile([128, 8], FP32, tag="b", name="bsb")

            # weights stream on the sync queue, first-needed first; chunk-0
            # inputs ride the gpsimd queue concurrently.
            nc.sync.dma_start(out=w8x[:], in_=w8xd[:])
            nc.sync.dma_start(out=w8h[:], in_=w8hd[:])
            nc.sync.dma_start(out=w8c[:], in_=w8cd[:])
            nc.sync.dma_start(out=w8t[:], in_=w8td[:])
            nc.sync.dma_start(out=w16[:], in_=w16d[:])
            nc.sync.dma_start(out=w16t[:], in_=w16td[:])
            nc.sync.dma_start(out=bsb[:], in_=biasd[:])

            # PE warm-up: tiny matmuls under the initial DMA fill get the
            # cost-model/HAM clock ramp out of the way before the real stream.
            wz = constp.tile([128, 64], FP16, tag="wz", name="wz")
            nc.vector.memset(wz[:], 0.0)
            pdum = pp.tile([128, NT], FP32, tag="ps0", name="pdum")
            for _ in range(28):
                nc.tensor.matmul(pdum[0:64, 0:64], wz[:, 0:64], wz[:, 0:64],
                                 start=True, stop=True)

            for n in range(nchunk):
                c0, c1 = n * NT, (n + 1) * NT

                # ---- input DMAs for this chunk ----
                x8 = zp.tile([128, 2, NT], FP8, tag="x8", name=f"x8_{n}")
                h8 = zp.tile([128, 2, NT], FP8, tag="h8", name=f"h8_{n}")
                c8 = zp.tile([128, 2, NT], FP8, tag="c8", name=f"c8_{n}")
                t8 = zp.tile([32, 2, NT], FP8, tag="t8", name=f"t8_{n}")
                z16 = zp.tile([128, 4, NT], FP16, tag="z16", name=f"z16_{n}")
                t16 = zp.tile([64, NT], FP16, tag="t16", name=f"t16_{n}")
                c16 = zp.tile([128, 2, NT], FP16, tag="c16", name=f"c16_{n}")
                if n == 0:
                    # keep the sync queue free for weights on the first chunk
                    q1 = q2 = nc.gpsimd
                else:
                    q1, q2 = nc.sync, nc.gpsimd
                q1.dma_start(out=x8[:], in_=x8d[:, :, c0:c1])
                q1.dma_start(out=h8[:], in_=h8d[:, :, c0:c1])
                q2.dma_start(out=z16[:], in_=z16d[:, :, c0:c1])
                q1.dma_start(out=c8[:], in_=c8d[:, :, c0:c1])
                q1.dma_start(out=t8[:], in_=t8d[:, :, c0:c1])
                q2.dma_start(out=t16[:], in_=t16d[:, c0:c1])
                q2.dma_start(out=c16[:], in_=c16d[:, :, c0:c1])

                # ---- matmuls ----
                ps = [None] * 8
                for m in range(8):
                    pst = pp.tile([128, NT], FP32, tag=f"ps{m}", name=f"ps{m}_{n}")
                    ps[m] = pst
                    if m in IFO_M:
                        i6 = IFO_M.index(m)
                        mc = slice(i6 * 128, (i6 + 1) * 128)
                        nc.tensor.matmul(pst[:], w8x[:, :, mc], x8[:],
                                         start=True, stop=False, perf_mode=DR)
                        nc.tensor.matmul(pst[:], w8h[:, :, mc], h8[:],
                                         start=False, stop=False, perf_mode=DR)
                        nc.tensor.matmul(pst[:], w8c[:, :, mc], c8[:],
                                         start=False, stop=False, perf_mode=DR)
                        nc.tensor.matmul(pst[:], w8t[:, :, mc], t8[:],
                                         start=False, stop=True, perf_mode=DR)
                    else:
                        m2 = G_M.index(m)
                        mc = slice(m2 * 128, (m2 + 1) * 128)
                        for b in range(4):
                            nc.tensor.matmul(pst[:], w16[:, b, mc], z16[:, b, :],
                                             start=(b == 0), stop=False)
                        nc.tensor.matmul(pst[:], w16t[:, mc], t16[:],
                                         start=False, stop=True)

                # ---- activations (bias + 1/WSCALE descale fused) ----
                def act(gt, m, half, fn, sc):
                    nc.scalar.activation(out=gt[:, half, :], in_=ps[m][:],
                                         func=fn, bias=bsb[:, m:m + 1], scale=sc)

                s8 = 1.0 / WSCALE
                ci = gp.tile([128, 2, NT], FP16, tag="ci", name=f"ci_{n}")
                cf = gp.tile([128, 2, NT], FP16, tag="cf", name=f"cf_{n}")
                tg = gp.tile([128, 2, NT], FP16, tag="tg", name=f"tg_{n}")
                co = gp.tile([128, 2, NT], FP16, tag="co", name=f"co_{n}")
                act(ci, 0, 0, SIG, s8); act(ci, 1, 1, SIG, s8)
                act(cf, 2, 0, SIG, s8); act(cf, 3, 1, SIG, s8)
                act(tg, 4, 0, TANH, 1.0); act(tg, 5, 1, TANH, 1.0)
                act(co, 6, 0, SIG, s8); act(co, 7, 1, SIG, s8)

                # ---- elementwise (fp16 on VectorE) ----
                t1 = gp.tile([128, 2, NT], FP16, tag="t1", name=f"t1_{n}")
                nc.vector.tensor_mul(t1[:], ci[:], tg[:])
                t2 = gp.tile([128, 2, NT], FP16, tag="t2", name=f"t2_{n}")
                nc.vector.tensor_mul(t2[:], cf[:], c16[:])
                cct = gp.tile([128, 2, NT], FP16, tag="cc", name=f"cc_{n}")
                nc.vector.tensor_add(cct[:], t1[:], t2[:])
                tcc = gp.tile([128, 2, NT], FP16, tag="tcc", name=f"tcc_{n}")
                nc.scalar.activation(out=tcc[:], in_=cct[:], func=TANH)
                cht = gp.tile([128, 2, NT], FP16, tag="ch", name=f"ch_{n}")
                nc.vector.tensor_mul(cht[:], co[:], tcc[:])

                nc.sync.dma_start(out=ccd[:, :, c0:c1], in_=cct[:])
                nc.gpsimd.dma_start(out=chd[:, :, c0:c1], in_=cht[:])

    nc.finalize()
    return nc


def _q8(a):
    return np.asarray(a, dtype=ml_dtypes.float8_e4m3)


def _prep_weights(inp):
    """Fused (1024, 832) weight -> DoubleRow fp8 lhsT blocks + fp16 g blocks."""
    Wf = np.zeros((1024, 832), np.float32)

    def put(g, blocks):
        r = g * 256
        for j, wb in enumerate(blocks):
            if wb is None:
                continue
            col = j * 256
            Wf[r:r + 256, col:col + wb.shape[1]] = wb

    put(0, [inp["w_ii"], inp["w_hi"], inp["w_ci"], inp["w_bi"]])
    put(1, [inp["w_if"], inp["w_hf"], inp["w_cf"], inp["w_bf"]])
    put(2, [inp["w_ic"], inp["w_hc"], None, inp["w_bc"]])
    put(3, [inp["w_io"], -inp["w_ho"], inp["w_co"], inp["w_bo"]])

    WT = np.ascontiguousarray(Wf.T)  # [832 K, 1024 M]

    # columns of the six i/f/o m-blocks, in IFO_M order
    mcols = np.concatenate([np.arange(m * 128, (m + 1) * 128) for m in IFO_M])
    WT8 = WT[:, mcols] * WSCALE      # [832, 768]

    def dr(k0, ki):  # -> [ki, 2, 768]
        blk = WT8[k0:k0 + 2 * ki].reshape(2, ki, 768).transpose(1, 0, 2)
        return _q8(np.ascontiguousarray(blk))

    w8x, w8h, w8c = dr(0, 128), dr(256, 128), dr(512, 128)
    w8t = dr(768, 32)

    gcols = np.arange(4 * 128, 6 * 128)
    WTg = WT[:, gcols]               # [832, 256]
    w16 = np.ascontiguousarray(
        WTg[0:512].reshape(4, 128, 256)).astype(np.float16)
    w16 = np.ascontiguousarray(w16.transpose(1, 0, 2))     # [128, 4, 256]
    w16t = np.ascontiguousarray(WTg[768:832]).astype(np.float16)  # [64, 256]

    bias_all = np.concatenate(
        [inp["bias_i"], inp["bias_f"], inp["bias_c"], inp["bias_o"]], axis=0
    ).reshape(8, 128)
    bias_host = np.ascontiguousarray(bias_all.T).astype(np.float32)  # [128, 8]
    return w8x, w8h, w8c, w8t, w16, w16t, bias_host


def _pairs(a, ki):
    """[2*ki, N] -> [ki, 2, N] with slot s = rows [s*ki, (s+1)*ki)."""
    return np.ascontiguousarray(a.reshape(2, ki, a.shape[1]).transpose(1, 0, 2))


def kernel(**inputs):
    global _PROGRAM, _LAST_RESULTS
    if _PROGRAM is None:
        _PROGRAM = _build_program()
    nc = _PROGRAM

    inp = {k: np.asarray(v, dtype=np.float32) for k, v in inputs.items()}
    w8x, w8h, w8c, w8t, w16, w16t, bias_host = _prep_weights(inp)

    xh = np.concatenate([inp["x"], inp["h"]], axis=0)     # [512, B]
    c = inp["c"]
    topic = inp["topic"]

    in_maps = []
    for i in range(NCORES):
        sl = slice(i * BS, (i + 1) * BS)
        xh_s, c_s, t_s = xh[:, sl], c[:, sl], topic[:, sl]
        in_maps.append({
            "x8": _q8(_pairs(xh_s[0:256], 128)),
            "h8": _q8(_pairs(xh_s[256:512], 128)),
            "c8": _q8(_pairs(c_s, 128)),
            "t8": _q8(_pairs(t_s, 32)),
            "z16": np.ascontiguousarray(
                xh_s.reshape(4, 128, BS).transpose(1, 0, 2)).astype(np.float16),
            "t16": np.ascontiguousarray(t_s).astype(np.float16),
            "c16": _pairs(c_s, 128).astype(np.float16),
            "w8x": w8x, "w8h": w8h, "w8c": w8c, "w8t": w8t,
            "w16": w16, "w16t": w16t, "biases": bias_host,
        })

    res = run_bass_kernel_spmd(
        nc, in_maps, list(range(NCORES)),
        trace=bool(os.environ.get("KERNEL_TRACE")),
    )
    _LAST_RESULTS = res

    def unpair(a):  # [128, 2, BS] -> [256, BS] fp32
        return np.asarray(a).transpose(1, 0, 2).reshape(256, BS).astype(np.float32)

    ch = np.concatenate([unpair(res.results[i]["ch"]) for i in range(NCORES)], axis=1)
    cc = np.concatenate([unpair(res.results[i]["cc"]) for i in range(NCORES)], axis=1)
    return np.stack([ch, cc], axis=0)


# revision 12
# speedup vs baseline: 1.9853x; 1.1340x over previous
"""Contextual LSTM cell on 8 Trainium2 NeuronCores.

Strategy (v2, fp8 DoubleRow):
  - Shard the batch dim (B=65536) across 8 cores (8192 each), replicate weights.
  - All gate matmuls fused into one (1024 x 832) @ (832 x B) matmul:
        rows:  [gate_i | gate_f | gate_c | gate_o]      (4 x 256)
        cols:  [x (256) | h (256) | c (256) | topic (64)]
    with -w_ho folded in and the (gate_c, c) block identically zero (skipped).
  - Precision split by gate (error budget measured vs fp32 reference):
      * sigmoid gates i/f/o: fp8 e4m3 DoubleRow matmuls. Each 128-row m-block
        needs only 4 matmul instructions (x-pair, h-pair, c-pair, topic-pair
        with Ki=32), each contracting 2x128 K rows per pass.
        Weights are scaled by 64 into e4m3's normal range; the 1/64 descale is
        folded into the ScalarE activation's `scale` operand.
      * tanh candidate gate: fp16 matmuls (the tanh path has unit slope and
        dominates the quantization error budget; fp8 there breaks 2e-2).
  - Elementwise cc/ch runs fp16 on VectorE; outputs ship fp16 and are upcast
    on the host. c is shipped fp16 for the elementwise path and e4m3 for the
    matmul path.
"""

import os
import numpy as np
import ml_dtypes

import concourse.bass as bass
import concourse.bacc as bacc
import concourse.mybir as mybir
from concourse.tile import TileContext
from concourse.bass_utils import run_bass_kernel_spmd

I, H, T, B = 256, 256, 64, 65536
NCORES = 8
BS = B // NCORES          # 8192 batch columns per core
NT = 512                  # columns per chunk (one PSUM bank of fp32)
NCHUNK = BS // NT         # 16

WSCALE = 64.0             # fp8 weight pre-scale (power of 2), descaled in ACT

FP8 = mybir.dt.float8e4
FP16 = mybir.dt.float16
FP32 = mybir.dt.float32
SIG = mybir.ActivationFunctionType.Sigmoid
TANH = mybir.ActivationFunctionType.Tanh
DR = mybir.MatmulPerfMode.DoubleRow

IFO_M = [0, 1, 2, 3, 6, 7]   # m-blocks of gates i, f, o (fp8 path)
G_M = [4, 5]                 # m-blocks of the tanh candidate gate (fp16 path)

_PROGRAM = None
_LAST_RESULTS = None  # for test harness introspection


def _build_program(nchunk=NCHUNK):
    nc = bacc.Bacc()

    # --- inputs (per-core shard), laid out host-side for direct 3D-AP DMA ---
    x8d = nc.declare_dram_parameter("x8", [128, 2, nchunk * NT], FP8, isOutput=False)
    h8d = nc.declare_dram_parameter("h8", [128, 2, nchunk * NT], FP8, isOutput=False)
    c8d = nc.declare_dram_parameter("c8", [128, 2, nchunk * NT], FP8, isOutput=False)
    t8d = nc.declare_dram_parameter("t8", [33, 2, nchunk * NT], FP8, isOutput=False)
    z16d = nc.declare_dram_parameter("z16", [128, 4, nchunk * NT], FP16, isOutput=False)
    t16d = nc.declare_dram_parameter("t16", [65, nchunk * NT], FP16, isOutput=False)
    c16d = nc.declare_dram_parameter("c16", [128, 2, nchunk * NT], FP16, isOutput=False)
    # weights (DoubleRow lhsT layout [Ki, 2, 6*128] for the six i/f/o m-blocks)
    w8xd = nc.declare_dram_parameter("w8x", [128, 2, 768], FP8, isOutput=False)
    w8hd = nc.declare_dram_parameter("w8h", [128, 2, 768], FP8, isOutput=False)
    w8cd = nc.declare_dram_parameter("w8c", [128, 2, 768], FP8, isOutput=False)
    w8td = nc.declare_dram_parameter("w8t", [33, 2, 768], FP8, isOutput=False)
    w16d = nc.declare_dram_parameter("w16", [128, 4, 256], FP16, isOutput=False)
    w16td = nc.declare_dram_parameter("w16t", [65, 256], FP16, isOutput=False)
    # outputs, fp16, [partition, half, col]
    ccd = nc.declare_dram_parameter("cc", [128, 2, nchunk * NT], FP16, isOutput=True)
    chd = nc.declare_dram_parameter("ch", [128, 2, nchunk * NT], FP16, isOutput=True)

    with TileContext(nc) as tc:
        with (
            tc.tile_pool(name="const", bufs=1) as constp,
            tc.tile_pool(name="zin", bufs=3) as zp,
            tc.tile_pool(name="gates", bufs=2) as gp,
            tc.tile_pool(name="psum", bufs=1, space="PSUM") as pp,
        ):
            w8x = constp.tile([128, 2, 768], FP8, tag="w8x", name="w8x")
            w8h = constp.tile([128, 2, 768], FP8, tag="w8h", name="w8h")
            w8c = constp.tile([128, 2, 768], FP8, tag="w8c", name="w8c")
            w8t = constp.tile([33, 2, 768], FP8, tag="w8t", name="w8t")
            w16 = constp.tile([128, 4, 256], FP16, tag="w16", name="w16")
            w16t = constp.tile([65, 256], FP16, tag="w16t", name="w16t")

            # weights stream on the sync queue, first-needed first; chunk-0
            # inputs ride the gpsimd queue concurrently.
            nc.sync.dma_start(out=w8x[:], in_=w8xd[:])
            nc.sync.dma_start(out=w8h[:], in_=w8hd[:])
            nc.sync.dma_start(out=w8c[:], in_=w8cd[:])
            nc.sync.dma_start(out=w8t[:], in_=w8td[:])
            nc.sync.dma_start(out=w16[:], in_=w16d[:])
            nc.sync.dma_start(out=w16t[:], in_=w16td[:])

            # PE warm-up: tiny matmuls under the initial DMA fill get the
            # cost-model/HAM clock ramp out of the way before the real stream.
            wz = constp.tile([128, 64], FP16, tag="wz", name="wz")
            nc.vector.memset(wz[:], 0.0)
            pdum = pp.tile([128, 2 * NT], FP32, tag="psA", name="pdum")
            for _ in range(28):
                nc.tensor.matmul(pdum[0:64, 0:64], wz[:, 0:64], wz[:, 0:64],
                                 start=True, stop=True)

            for n in range(nchunk):
                c0, c1 = n * NT, (n + 1) * NT

                # ---- input DMAs for this chunk ----
                x8 = zp.tile([128, 2, NT], FP8, tag="x8", name=f"x8_{n}")
                h8 = zp.tile([128, 2, NT], FP8, tag="h8", name=f"h8_{n}")
                c8 = zp.tile([128, 2, NT], FP8, tag="c8", name=f"c8_{n}")
                t8 = zp.tile([33, 2, NT], FP8, tag="t8", name=f"t8_{n}")
                z16 = zp.tile([128, 4, NT], FP16, tag="z16", name=f"z16_{n}")
                t16 = zp.tile([65, NT], FP16, tag="t16", name=f"t16_{n}")
                c16 = zp.tile([128, 2, NT], FP16, tag="c16", name=f"c16_{n}")
                if n == 0:
                    # keep the sync queue free for weights on the first chunk
                    q1 = q2 = nc.gpsimd
                else:
                    q1, q2 = nc.sync, nc.gpsimd
                q1.dma_start(out=x8[:], in_=x8d[:, :, c0:c1])
                q1.dma_start(out=h8[:], in_=h8d[:, :, c0:c1])
                q2.dma_start(out=z16[:], in_=z16d[:, :, c0:c1])
                q1.dma_start(out=c8[:], in_=c8d[:, :, c0:c1])
                q1.dma_start(out=t8[:], in_=t8d[:, :, c0:c1])
                q2.dma_start(out=t16[:], in_=t16d[:, c0:c1])
                q2.dma_start(out=c16[:], in_=c16d[:, :, c0:c1])

                # ---- matmuls (bias folded in as a ones-row on the topic MM) ----
                # One [128, 1024] PSUM tile (2 adjacent banks) per gate; the
                # two 128-row m-blocks land in its column halves so a single
                # ScalarE activation covers the whole gate.
                s8 = 1.0 / WSCALE
                ci = gp.tile([128, 2, NT], FP16, tag="ci", name=f"ci_{n}")
                cf = gp.tile([128, 2, NT], FP16, tag="cf", name=f"cf_{n}")
                tg = gp.tile([128, 2, NT], FP16, tag="tg", name=f"tg_{n}")
                co = gp.tile([128, 2, NT], FP16, tag="co", name=f"co_{n}")
                gates = [("A", ci, SIG, s8), ("B", cf, SIG, s8),
                         ("C", tg, TANH, 1.0), ("D", co, SIG, s8)]
                for g, (gtag, gt, fn, sc) in enumerate(gates):
                    pst = pp.tile([128, 2 * NT], FP32, tag=f"ps{gtag}",
                                  name=f"ps{gtag}_{n}")
                    for half in range(2):
                        m = 2 * g + half
                        out = pst[:, half * NT:(half + 1) * NT]
                        if m in IFO_M:
                            i6 = IFO_M.index(m)
                            mc = slice(i6 * 128, (i6 + 1) * 128)
                            nc.tensor.matmul(out, w8x[:, :, mc], x8[:],
                                             start=True, stop=False, perf_mode=DR)
                            nc.tensor.matmul(out, w8h[:, :, mc], h8[:],
                                             start=False, stop=False, perf_mode=DR)
                            nc.tensor.matmul(out, w8c[:, :, mc], c8[:],
                                             start=False, stop=False, perf_mode=DR)
                            nc.tensor.matmul(out, w8t[:, :, mc], t8[:],
                                             start=False, stop=True, perf_mode=DR)
                        else:
                            m2 = G_M.index(m)
                            mc = slice(m2 * 128, (m2 + 1) * 128)
                            for b in range(4):
                                nc.tensor.matmul(out, w16[:, b, mc], z16[:, b, :],
                                                 start=(b == 0), stop=False)
                            nc.tensor.matmul(out, w16t[:, mc], t16[:],
                                             start=False, stop=True)
                    nc.scalar.activation(out=gt[:], in_=pst[:], func=fn, scale=sc)

                # ---- elementwise (fp16 on VectorE) ----
                t1 = gp.tile([128, 2, NT], FP16, tag="t1", name=f"t1_{n}")
                nc.vector.tensor_mul(t1[:], ci[:], tg[:])
                t2 = gp.tile([128, 2, NT], FP16, tag="t2", name=f"t2_{n}")
                nc.vector.tensor_mul(t2[:], cf[:], c16[:])
                cct = gp.tile([128, 2, NT], FP16, tag="cc", name=f"cc_{n}")
                nc.vector.tensor_add(cct[:], t1[:], t2[:])
                tcc = gp.tile([128, 2, NT], FP16, tag="tcc", name=f"tcc_{n}")
                nc.scalar.activation(out=tcc[:], in_=cct[:], func=TANH)
                cht = gp.tile([128, 2, NT], FP16, tag="ch", name=f"ch_{n}")
                nc.vector.tensor_mul(cht[:], co[:], tcc[:])

                nc.sync.dma_start(out=ccd[:, :, c0:c1], in_=cct[:])
                nc.gpsimd.dma_start(out=chd[:, :, c0:c1], in_=cht[:])

    nc.finalize()
    return nc


def _q8(a):
    return np.asarray(a, dtype=ml_dtypes.float8_e4m3)


def _prep_weights(inp):
    """Fused (1024, 832) weight -> DoubleRow fp8 lhsT blocks + fp16 g blocks."""
    Wf = np.zeros((1024, 832), np.float32)

    def put(g, blocks):
        r = g * 256
        for j, wb in enumerate(blocks):
            if wb is None:
                continue
            col = j * 256
            Wf[r:r + 256, col:col + wb.shape[1]] = wb

    put(0, [inp["w_ii"], inp["w_hi"], inp["w_ci"], inp["w_bi"]])
    put(1, [inp["w_if"], inp["w_hf"], inp["w_cf"], inp["w_bf"]])
    put(2, [inp["w_ic"], inp["w_hc"], None, inp["w_bc"]])
    put(3, [inp["w_io"], -inp["w_ho"], inp["w_co"], inp["w_bo"]])

    WT = np.ascontiguousarray(Wf.T)  # [832 K, 1024 M]

    # columns of the six i/f/o m-blocks, in IFO_M order
    mcols = np.concatenate([np.arange(m * 128, (m + 1) * 128) for m in IFO_M])
    WT8 = WT[:, mcols] * WSCALE      # [832, 768]

    bias_all = np.concatenate(
        [inp["bias_i"], inp["bias_f"], inp["bias_c"], inp["bias_o"]], axis=0
    )[:, 0]                          # [1024]

    def dr(k0, ki):  # -> [ki, 2, 768]
        blk = WT8[k0:k0 + 2 * ki].reshape(2, ki, 768).transpose(1, 0, 2)
        return _q8(np.ascontiguousarray(blk))

    w8x, w8h, w8c = dr(0, 128), dr(256, 128), dr(512, 128)
    # topic block gets an extra Ki row carrying the gate bias (ones-row input)
    w8t = np.zeros((33, 2, 768), np.float32)
    w8t[0:32] = WT8[768:832].reshape(2, 32, 768).transpose(1, 0, 2)
    w8t[32, 0] = bias_all[mcols] * WSCALE
    w8t = _q8(w8t)

    gcols = np.arange(4 * 128, 6 * 128)
    WTg = WT[:, gcols]               # [832, 256]
    w16 = np.ascontiguousarray(
        WTg[0:512].reshape(4, 128, 256)).astype(np.float16)
    w16 = np.ascontiguousarray(w16.transpose(1, 0, 2))     # [128, 4, 256]
    w16t = np.zeros((65, 256), np.float32)
    w16t[0:64] = WTg[768:832]
    w16t[64] = bias_all[gcols]
    w16t = w16t.astype(np.float16)   # [65, 256]
    return w8x, w8h, w8c, w8t, w16, w16t


def _pairs(a, ki):
    """[2*ki, N] -> [ki, 2, N] with slot s = rows [s*ki, (s+1)*ki)."""
    return np.ascontiguousarray(a.reshape(2, ki, a.shape[1]).transpose(1, 0, 2))


def kernel(**inputs):
    global _PROGRAM, _LAST_RESULTS
    if _PROGRAM is None:
        _PROGRAM = _build_program()
    nc = _PROGRAM

    inp = {k: np.asarray(v, dtype=np.float32) for k, v in inputs.items()}
    w8x, w8h, w8c, w8t, w16, w16t = _prep_weights(inp)

    xh = np.concatenate([inp["x"], inp["h"]], axis=0)     # [512, B]
    c = inp["c"]
    topic = inp["topic"]

    in_maps = []
    for i in range(NCORES):
        sl = slice(i * BS, (i + 1) * BS)
        xh_s, c_s, t_s = xh[:, sl], c[:, sl], topic[:, sl]
        t8s = np.ones((33, 2, BS), np.float32)
        t8s[0:32] = t_s.reshape(2, 32, BS).transpose(1, 0, 2)
        t16s = np.ones((65, BS), np.float32)
        t16s[0:64] = t_s
        in_maps.append({
            "x8": _q8(_pairs(xh_s[0:256], 128)),
            "h8": _q8(_pairs(xh_s[256:512], 128)),
            "c8": _q8(_pairs(c_s, 128)),
            "t8": _q8(t8s),
            "z16": np.ascontiguousarray(
                xh_s.reshape(4, 128, BS).transpose(1, 0, 2)).astype(np.float16),
            "t16": t16s.astype(np.float16),
            "c16": _pairs(c_s, 128).astype(np.float16),
            "w8x": w8x, "w8h": w8h, "w8c": w8c, "w8t": w8t,
            "w16": w16, "w16t": w16t,
        })

    res = run_bass_kernel_spmd(
        nc, in_maps, list(range(NCORES)),
        trace=bool(os.environ.get("KERNEL_TRACE")),
    )
    _LAST_RESULTS = res

    def unpair(a):  # [128, 2, BS] -> [256, BS] fp32
        return np.asarray(a).transpose(1, 0, 2).reshape(256, BS).astype(np.float32)

    ch = np.concatenate([unpair(res.results[i]["ch"]) for i in range(NCORES)], axis=1)
    cc = np.concatenate([unpair(res.results[i]["cc"]) for i in range(NCORES)], axis=1)
    return np.stack([ch, cc], axis=0)


# revision 27
# speedup vs baseline: 2.0121x; 1.0135x over previous
"""Contextual LSTM cell on 8 Trainium2 NeuronCores.

Strategy (v2, fp8 DoubleRow):
  - Shard the batch dim (B=65536) across 8 cores (8192 each), replicate weights.
  - All gate matmuls fused into one (1024 x 832) @ (832 x B) matmul:
        rows:  [gate_i | gate_f | gate_c | gate_o]      (4 x 256)
        cols:  [x (256) | h (256) | c (256) | topic (64)]
    with -w_ho folded in and the (gate_c, c) block identically zero (skipped).
  - Precision split by gate (error budget measured vs fp32 reference):
      * sigmoid gates i/f/o: fp8 e4m3 DoubleRow matmuls. Each 128-row m-block
        needs only 4 matmul instructions (x-pair, h-pair, c-pair, topic-pair
        with Ki=32), each contracting 2x128 K rows per pass.
        Weights are scaled by 64 into e4m3's normal range; the 1/64 descale is
        folded into the ScalarE activation's `scale` operand.
      * tanh candidate gate: fp16 matmuls (the tanh path has unit slope and
        dominates the quantization error budget; fp8 there breaks 2e-2).
  - Elementwise cc/ch runs fp16 on VectorE; outputs ship fp16 and are upcast
    on the host. c is shipped fp16 for the elementwise path and e4m3 for the
    matmul path.
"""

import os
import numpy as np
import ml_dtypes

import concourse.bass as bass
import concourse.bacc as bacc
import concourse.mybir as mybir
from concourse.tile import TileContext
from concourse.bass_utils import run_bass_kernel_spmd

I, H, T, B = 256, 256, 64, 65536
NCORES = 8
BS = B // NCORES          # 8192 batch columns per core
NT = 512                  # columns per chunk (one PSUM bank of fp32)
NCHUNK = BS // NT         # 16

WSCALE = 64.0             # fp8 weight pre-scale (power of 2), descaled in ACT

FP8 = mybir.dt.float8e4
FP16 = mybir.dt.float16
FP32 = mybir.dt.float32
SIG = mybir.ActivationFunctionType.Sigmoid
TANH = mybir.ActivationFunctionType.Tanh
DR = mybir.MatmulPerfMode.DoubleRow

IFO_M = [0, 1, 2, 3, 6, 7]   # m-blocks of gates i, f, o (fp8 path)
G_M = [4, 5]                 # m-blocks of the tanh candidate gate (fp16 path)

_PROGRAM = None
_LAST_RESULTS = None  # for test harness introspection


def _build_program(nchunk=NCHUNK):
    nc = bacc.Bacc()

    # --- inputs (per-core shard), laid out host-side for direct 3D-AP DMA ---
    x8d = nc.declare_dram_parameter("x8", [128, 2, nchunk * NT], FP8, isOutput=False)
    h8d = nc.declare_dram_parameter("h8", [128, 2, nchunk * NT], FP8, isOutput=False)
    c8d = nc.declare_dram_parameter("c8", [128, 2, nchunk * NT], FP8, isOutput=False)
    t8d = nc.declare_dram_parameter("t8", [33, 2, nchunk * NT], FP8, isOutput=False)
    z16d = nc.declare_dram_parameter("z16", [128, 4, nchunk * NT], FP16, isOutput=False)
    t16d = nc.declare_dram_parameter("t16", [65, nchunk * NT], FP16, isOutput=False)
    c16d = nc.declare_dram_parameter("c16", [128, 2, nchunk * NT], FP16, isOutput=False)
    # weights (DoubleRow lhsT layout [Ki, 2, 6*128] for the six i/f/o m-blocks)
    w8xd = nc.declare_dram_parameter("w8x", [128, 2, 768], FP8, isOutput=False)
    w8hd = nc.declare_dram_parameter("w8h", [128, 2, 768], FP8, isOutput=False)
    w8cd = nc.declare_dram_parameter("w8c", [128, 2, 768], FP8, isOutput=False)
    w8td = nc.declare_dram_parameter("w8t", [33, 2, 768], FP8, isOutput=False)
    w16d = nc.declare_dram_parameter("w16", [128, 4, 256], FP16, isOutput=False)
    w16td = nc.declare_dram_parameter("w16t", [65, 256], FP16, isOutput=False)
    # outputs, fp16, [partition, half, col]
    ccd = nc.declare_dram_parameter("cc", [128, 2, nchunk * NT], FP16, isOutput=True)
    chd = nc.declare_dram_parameter("ch", [128, 2, nchunk * NT], FP16, isOutput=True)

    with TileContext(nc) as tc:
        with (
            tc.tile_pool(name="const", bufs=1) as constp,
            tc.tile_pool(name="zin", bufs=3) as zp,
            tc.tile_pool(name="gates", bufs=2) as gp,
            tc.tile_pool(name="psum", bufs=1, space="PSUM") as pp,
        ):
            w8x = constp.tile([128, 2, 768], FP8, tag="w8x", name="w8x")
            w8h = constp.tile([128, 2, 768], FP8, tag="w8h", name="w8h")
            w8c = constp.tile([128, 2, 768], FP8, tag="w8c", name="w8c")
            w8t = constp.tile([33, 2, 768], FP8, tag="w8t", name="w8t")
            w16 = constp.tile([128, 4, 256], FP16, tag="w16", name="w16")
            w16t = constp.tile([65, 256], FP16, tag="w16t", name="w16t")

            # weights stream on the sync queue, first-needed first; chunk-0
            # inputs ride the gpsimd queue concurrently.
            nc.sync.dma_start(out=w8x[:], in_=w8xd[:])
            nc.sync.dma_start(out=w8h[:], in_=w8hd[:])
            nc.sync.dma_start(out=w8c[:], in_=w8cd[:])
            nc.sync.dma_start(out=w8t[:], in_=w8td[:])
            nc.sync.dma_start(out=w16[:], in_=w16d[:])
            nc.sync.dma_start(out=w16t[:], in_=w16td[:])

            # PE warm-up: tiny matmuls under the initial DMA fill get the
            # cost-model/HAM clock ramp out of the way before the real stream.
            wz = constp.tile([128, 128], FP16, tag="wz", name="wz")
            nc.vector.memset(wz[:], 0.0)
            pdum = pp.tile([128, 2, NT], FP32, tag="psA", name="pdum")
            for _ in range(30):
                nc.tensor.matmul(pdum[0:64, 0, 0:128], wz[:, 0:64], wz[:],
                                 start=True, stop=True)

            # taper: small chunks at the start (pipeline fill) and end (drain)
            total = nchunk * NT
            sizes = [NT // 2, NT // 2] + [NT] * (nchunk - 1)
            assert sum(sizes) == total
            offs = [sum(sizes[:i]) for i in range(len(sizes))]
            for n, (c0, nt) in enumerate(zip(offs, sizes)):
                c1 = c0 + nt

                # ---- input DMAs for this chunk ----
                x8 = zp.tile([128, 2, NT], FP8, tag="x8", name=f"x8_{n}")
                h8 = zp.tile([128, 2, NT], FP8, tag="h8", name=f"h8_{n}")
                c8 = zp.tile([128, 2, NT], FP8, tag="c8", name=f"c8_{n}")
                t8 = zp.tile([33, 2, NT], FP8, tag="t8", name=f"t8_{n}")
                z16 = zp.tile([128, 4, NT], FP16, tag="z16", name=f"z16_{n}")
                t16 = zp.tile([65, NT], FP16, tag="t16", name=f"t16_{n}")
                c16 = zp.tile([128, 2, NT], FP16, tag="c16", name=f"c16_{n}")
                if n == 0:
                    # keep the sync queue free for weights on the first chunk
                    q1 = q2 = nc.gpsimd
                else:
                    q1, q2 = nc.sync, nc.gpsimd
                q1.dma_start(out=x8[:, :, 0:nt], in_=x8d[:, :, c0:c1])
                q1.dma_start(out=h8[:, :, 0:nt], in_=h8d[:, :, c0:c1])
                q2.dma_start(out=z16[:, :, 0:nt], in_=z16d[:, :, c0:c1])
                q1.dma_start(out=c8[:, :, 0:nt], in_=c8d[:, :, c0:c1])
                q1.dma_start(out=t8[:, :, 0:nt], in_=t8d[:, :, c0:c1])
                q2.dma_start(out=t16[:, 0:nt], in_=t16d[:, c0:c1])
                q2.dma_start(out=c16[:, :, 0:nt], in_=c16d[:, :, c0:c1])

                # ---- matmuls (bias folded in as a ones-row on the topic MM) ----
                # One [128, 1024] PSUM tile (2 adjacent banks) per gate; the
                # two 128-row m-blocks land in its column halves so a single
                # ScalarE activation covers the whole gate.
                s8 = 1.0 / WSCALE
                ci = gp.tile([128, 2, NT], FP16, tag="ci", name=f"ci_{n}")
                cf = gp.tile([128, 2, NT], FP16, tag="cf", name=f"cf_{n}")
                tg = gp.tile([128, 2, NT], FP16, tag="tg", name=f"tg_{n}")
                co = gp.tile([128, 2, NT], FP16, tag="co", name=f"co_{n}")
                gates = [("A", ci, SIG, s8), ("B", cf, SIG, s8),
                         ("C", tg, TANH, 1.0), ("D", co, SIG, s8)]
                for g, (gtag, gt, fn, sc) in enumerate(gates):
                    pst = pp.tile([128, 2, NT], FP32, tag=f"ps{gtag}",
                                  name=f"ps{gtag}_{n}")
                    for half in range(2):
                        m = 2 * g + half
                        out = pst[:, half, 0:nt]
                        if m in IFO_M:
                            i6 = IFO_M.index(m)
                            mc = slice(i6 * 128, (i6 + 1) * 128)
                            nc.tensor.matmul(out, w8x[:, :, mc], x8[:, :, 0:nt],
                                             start=True, stop=False, perf_mode=DR)
                            nc.tensor.matmul(out, w8h[:, :, mc], h8[:, :, 0:nt],
                                             start=False, stop=False, perf_mode=DR)
                            nc.tensor.matmul(out, w8c[:, :, mc], c8[:, :, 0:nt],
                                             start=False, stop=False, perf_mode=DR)
                            nc.tensor.matmul(out, w8t[:, :, mc], t8[:, :, 0:nt],
                                             start=False, stop=True, perf_mode=DR)
                        else:
                            m2 = G_M.index(m)
                            mc = slice(m2 * 128, (m2 + 1) * 128)
                            for b in range(4):
                                nc.tensor.matmul(out, w16[:, b, mc], z16[:, b, 0:nt],
                                                 start=(b == 0), stop=False)
                            nc.tensor.matmul(out, w16t[:, mc], t16[:, 0:nt],
                                             start=False, stop=True)
                    nc.scalar.activation(out=gt[:, :, 0:nt], in_=pst[:, :, 0:nt],
                                         func=fn, scale=sc)

                # ---- elementwise (fp16 on VectorE) ----
                t1 = gp.tile([128, 2, NT], FP16, tag="t1", name=f"t1_{n}")
                nc.vector.tensor_mul(t1[:, :, 0:nt], ci[:, :, 0:nt], tg[:, :, 0:nt])
                t2 = gp.tile([128, 2, NT], FP16, tag="t2", name=f"t2_{n}")
                nc.vector.tensor_mul(t2[:, :, 0:nt], cf[:, :, 0:nt], c16[:, :, 0:nt])
                cct = gp.tile([128, 2, NT], FP16, tag="cc", name=f"cc_{n}")
                nc.vector.tensor_add(cct[:, :, 0:nt], t1[:, :, 0:nt], t2[:, :, 0:nt])
                tcc = gp.tile([128, 2, NT], FP16, tag="tcc", name=f"tcc_{n}")
                nc.scalar.activation(out=tcc[:, :, 0:nt], in_=cct[:, :, 0:nt], func=TANH)
                cht = gp.tile([128, 2, NT], FP16, tag="ch", name=f"ch_{n}")
                nc.vector.tensor_mul(cht[:, :, 0:nt], co[:, :, 0:nt], tcc[:, :, 0:nt])
                nc.sync.dma_start(out=ccd[:, :, c0:c1], in_=cct[:, :, 0:nt])
                nc.gpsimd.dma_start(out=chd[:, :, c0:c1], in_=cht[:, :, 0:nt])

    nc.finalize()
    return nc


def _q8(a):
    return np.asarray(a, dtype=ml_dtypes.float8_e4m3)


def _prep_weights(inp):
    """Fused (1024, 832) weight -> DoubleRow fp8 lhsT blocks + fp16 g blocks."""
    Wf = np.zeros((1024, 832), np.float32)

    def put(g, blocks):
        r = g * 256
        for j, wb in enumerate(blocks):
            if wb is None:
                continue
            col = j * 256
            Wf[r:r + 256, col:col + wb.shape[1]] = wb

    put(0, [inp["w_ii"], inp["w_hi"], inp["w_ci"], inp["w_bi"]])
    put(1, [inp["w_if"], inp["w_hf"], inp["w_cf"], inp["w_bf"]])
    put(2, [inp["w_ic"], inp["w_hc"], None, inp["w_bc"]])
    put(3, [inp["w_io"], -inp["w_ho"], inp["w_co"], inp["w_bo"]])

    WT = np.ascontiguousarray(Wf.T)  # [832 K, 1024 M]

    # columns of the six i/f/o m-blocks, in IFO_M order
    mcols = np.concatenate([np.arange(m * 128, (m + 1) * 128) for m in IFO_M])
    WT8 = WT[:, mcols] * WSCALE      # [832, 768]

    bias_all = np.concatenate(
        [inp["bias_i"], inp["bias_f"], inp["bias_c"], inp["bias_o"]], axis=0
    )[:, 0]                          # [1024]

    def dr(k0, ki):  # -> [ki, 2, 768]
        blk = WT8[k0:k0 + 2 * ki].reshape(2, ki, 768).transpose(1, 0, 2)
        return _q8(np.ascontiguousarray(blk))

    w8x, w8h, w8c = dr(0, 128), dr(256, 128), dr(512, 128)
    # topic block gets an extra Ki row carrying the gate bias (ones-row input)
    w8t = np.zeros((33, 2, 768), np.float32)
    w8t[0:32] = WT8[768:832].reshape(2, 32, 768).transpose(1, 0, 2)
    w8t[32, 0] = bias_all[mcols] * WSCALE
    w8t = _q8(w8t)

    gcols = np.arange(4 * 128, 6 * 128)
    WTg = WT[:, gcols]               # [832, 256]
    w16 = np.ascontiguousarray(
        WTg[0:512].reshape(4, 128, 256)).astype(np.float16)
    w16 = np.ascontiguousarray(w16.transpose(1, 0, 2))     # [128, 4, 256]
    w16t = np.zeros((65, 256), np.float32)
    w16t[0:64] = WTg[768:832]
    w16t[64] = bias_all[gcols]
    w16t = w16t.astype(np.float16)   # [65, 256]
    return w8x, w8h, w8c, w8t, w16, w16t


def _pairs(a, ki):
    """[2*ki, N] -> [ki, 2, N] with slot s = rows [s*ki, (s+1)*ki)."""
    return np.ascontiguousarray(a.reshape(2, ki, a.shape[1]).transpose(1, 0, 2))


def kernel(**inputs):
    global _PROGRAM, _LAST_RESULTS
    if _PROGRAM is None:
        _PROGRAM = _build_program()
    nc = _PROGRAM

    inp = {k: np.asarray(v, dtype=np.float32) for k, v in inputs.items()}
    w8x, w8h, w8c, w8t, w16, w16t = _prep_weights(inp)

    xh = np.concatenate([inp["x"], inp["h"]], axis=0)     # [512, B]
    c = inp["c"]
    topic = inp["topic"]

    in_maps = []
    for i in range(NCORES):
        sl = slice(i * BS, (i + 1) * BS)
        xh_s, c_s, t_s = xh[:, sl], c[:, sl], topic[:, sl]
        t8s = np.ones((33, 2, BS), np.float32)
        t8s[0:32] = t_s.reshape(2, 32, BS).transpose(1, 0, 2)
        t16s = np.ones((65, BS), np.float32)
        t16s[0:64] = t_s
        in_maps.append({
            "x8": _q8(_pairs(xh_s[0:256], 128)),
            "h8": _q8(_pairs(xh_s[256:512], 128)),
            "c8": _q8(_pairs(c_s, 128)),
            "t8": _q8(t8s),
            "z16": np.ascontiguousarray(
                xh_s.reshape(4, 128, BS).transpose(1, 0, 2)).astype(np.float16),
            "t16": t16s.astype(np.float16),
            "c16": _pairs(c_s, 128).astype(np.float16),
            "w8x": w8x, "w8h": w8h, "w8c": w8c, "w8t": w8t,
            "w16": w16, "w16t": w16t,
        })

    res = run_bass_kernel_spmd(
        nc, in_maps, list(range(NCORES)),
        trace=bool(os.environ.get("KERNEL_TRACE")),
    )
    _LAST_RESULTS = res

    def unpair(a):  # [128, 2, BS] -> [256, BS] fp32
        return np.asarray(a).transpose(1, 0, 2).reshape(256, BS).astype(np.float32)

    ch = np.concatenate([unpair(res.results[i]["ch"]) for i in range(NCORES)], axis=1)
    cc = np.concatenate([unpair(res.results[i]["cc"]) for i in range(NCORES)], axis=1)
    return np.stack([ch, cc], axis=0)
